# revision 2
# baseline (speedup 1.0000x reference)
"""Trainium2 Bass kernel for DicGaussianRBF.

out = concat([ones(N,1), data, exp(-5 * ||data - centers||^2)], axis=-1)
with data [65536, 256] f32, centers [2048, 256] f32 -> out [65536, 2305] f32.

Data-parallel over N across 8 NeuronCores; centers replicated. The device
computes only the RBF block [N/8, K] in bf16; the host assembles the final
f32 output (ones column and data pass-through are pure input marshaling).
Inputs are pre-cast to bf16 on the host: the device matmul consumes bf16
anyway, so uploading bf16 halves the HBM read without changing numerics.

Per core (8192 rows, 64 row-blocks of 128):

  setup: centers staged once; cT (bf16, [d,k] layout) via DMA-xbar
  transposes; c2 = ||c||^2 via ones-matmul of cT^2; e5rep = exp(-5*c2)
  replicated to [128, K] via a PE broadcast matmul.

  steady state (skewed by PRE row-blocks):
    - input staged 8 row-blocks (512 KB bf16) per SWDGE DMA.
    - per row-block: DVE computes bias = -5*||x||^2 in one
      scalar_tensor_tensor; two DMA-xbar transposes produce dT [d, n];
      8 matmuls (2 contraction chunks x 4 psum banks) accumulate
      psum = x.c over a full [128, 2048] 4-bank psum tile; ScalarE
      evaluates exp(10*psum + bias) at FD=2048 into a bf16 tile
      (exp(10xc - 5x^2)); DVE multiplies by e5rep -> rbf bf16; HWDGE
      DMAs it out.

Factorization note: exp(-5r^2) = exp(10xc - 5x^2) * exp(-5c2). The first
factor can overflow f32 only if 10xc - 5x^2 > 88, which requires some
||c||^2 > 17.6 with x aligned to c; for such adversarial inputs the fused
single-exp form (baseline) is more robust. For data in the reference's
regime the margin is > e^40.
"""

import sys

for _p in ("/opt/trn_rl_repo",):
    if _p not in sys.path:
        sys.path.insert(0, _p)

import numpy as np
import ml_dtypes

import concourse.bass as bass
import concourse.tile as tile
from concourse import bacc, mybir
from concourse import bass_utils

N, D, K = 65536, 256, 2048
NCORES = 8
N_LOC = N // NCORES          # 8192 rows per core
OUT_W = 1 + D + K            # 2305
RB = N_LOC // 128            # 64 row blocks per core
SB = 8                       # row blocks per input staging DMA
PRE = 3                      # transpose pipeline lookahead (row blocks)
S = 5.0

FP32 = mybir.dt.float32
BF16 = mybir.dt.bfloat16
Act = mybir.ActivationFunctionType
MULT = mybir.AluOpType.mult

_cached_nc = None


def _build():
    nc = bacc.Bacc(
        "TRN2",
        target_bir_lowering=False,
        debug=False,
        enable_asserts=False,
        num_devices=NCORES,
    )
    data_ap = nc.dram_tensor("data", [N_LOC, D], BF16, kind="ExternalInput").ap()
    cent_ap = nc.dram_tensor("centers", [K, D], BF16, kind="ExternalInput").ap()
    out_ap = nc.dram_tensor("rbf", [N_LOC, K], BF16, kind="ExternalOutput").ap()

    with tile.TileContext(nc) as tc:
        with (
            tc.tile_pool(name="const", bufs=1) as const,
            tc.tile_pool(name="cload", bufs=1) as cload,
            tc.tile_pool(name="dinp", bufs=4) as dinp,
            tc.tile_pool(name="rbfp", bufs=3) as rbfp,
            tc.tile_pool(name="prawp", bufs=3) as prawp,
            tc.tile_pool(name="dtp", bufs=6) as dtp,
            tc.tile_pool(name="scrp", bufs=3) as scrp,
            tc.tile_pool(name="biasp", bufs=8) as biasp,
            tc.tile_pool(name="psmm", bufs=2, space="PSUM") as psmm,
        ):
            ones_col = const.tile([128, 1], BF16)
            nc.vector.memset(ones_col[:], 1.0)
            ones_row = const.tile([1, 128], BF16)
            nc.vector.memset(ones_row[:], 1.0)
            warm = const.tile([128, 512], BF16)
            nc.vector.memset(warm[:], 0.0)

            # dummy matmuls engage the PE HAM clock-gate so the real matmul
            # stream starts at 2.4 GHz
            pw = psmm.tile([128, 2048], FP32, tag="mm", name="pw")

            def warm_mms(n):
                for _ in range(n):
                    nc.tensor.matmul(pw[0:1, 0:512], ones_col[:], warm[:], start=True, stop=True)

            # centers staged as [128, 16, 256] (16 k-tiles of 128)
            cstage = cload.tile([128, (K // 128) * D], BF16)
            cst3 = cstage[:].rearrange("p (t d) -> p t d", d=D)

            din_tiles = {}

            def load_super_block(sb):
                din = dinp.tile([128, SB * D], BF16, tag="din", name="din")
                din_tiles[sb] = din
                din3 = din[:].rearrange("p (r c) -> p r c", c=D)
                src = data_ap[sb * SB * 128:(sb + 1) * SB * 128, :].rearrange(
                    "(r p) d -> p r d", p=128
                )
                nc.gpsimd.dma_start(din3[:, :, :], src)

            for c in range(4):
                csrc = cent_ap[c * 512:(c + 1) * 512, :].rearrange(
                    "(t p) d -> p t d", p=128
                )
                nc.scalar.dma_start(cst3[:, c * 4:(c + 1) * 4, :], csrc)
                if c == 0:
                    load_super_block(0)
            load_super_block(1)

            # cT: [d, k] layout, bf16. cT0 = dims 0:128, cT1 = dims 128:256.
            cT0 = const.tile([128, K], BF16)
            cT1 = const.tile([128, K], BF16)
            for t in range(K // 128):
                ks = slice(t * 128, (t + 1) * 128)
                nc.sync.dma_start_transpose(cT0[:, ks], cst3[:, t, 0:128])
                nc.sync.dma_start_transpose(cT1[:, ks], cst3[:, t, 128:256])

            warm_mms(10)

            # c2 = ||c||^2 row via ones-matmul of cT^2; e5row = exp(-5*c2)
            sq0 = cload.tile([128, K], BF16, name="sq0")
            sq1 = cload.tile([128, K], BF16, name="sq1")
            nc.vector.tensor_mul(sq0[:], cT0[:], cT0[:])
            nc.vector.tensor_mul(sq1[:], cT1[:], cT1[:])
            pc2 = psmm.tile([128, 2048], FP32, tag="mm", name="pc2")
            for j in range(4):
                ks = slice(j * 512, (j + 1) * 512)
                nc.tensor.matmul(pc2[0:1, ks], ones_col[:], sq0[:, ks], start=True, stop=False)
                nc.tensor.matmul(pc2[0:1, ks], ones_col[:], sq1[:, ks], start=False, stop=True)
            e5row = const.tile([1, K], BF16)
            nc.scalar.activation(e5row[:], pc2[0:1, :], Act.Exp, scale=-S)

            # replicate exp(-5*c2) across 128 partitions via PE broadcast
            pb = psmm.tile([128, 2048], FP32, tag="mm", name="pb")
            for j in range(4):
                ks = slice(j * 512, (j + 1) * 512)
                nc.tensor.matmul(pb[:, ks], ones_row[:], e5row[0:1, ks], start=True, stop=True)
            e5rep = const.tile([128, K], BF16)
            nc.vector.tensor_copy(e5rep[:], pb[:])

            stage = {}
            for step in range(RB + PRE):
                # ---- front of the pipe: stage input, bias, xbar transpose
                rb = step
                if rb < RB:
                    if rb % SB == 0 and rb // SB + 2 < RB // SB:
                        load_super_block(rb // SB + 2)
                    din = din_tiles[rb // SB]
                    b = rb % SB
                    dcol = din[:, b * D:(b + 1) * D]

                    scratch = scrp.tile([128, D], BF16, tag="scr")
                    bias = biasp.tile([128, 1], FP32, tag="bias")
                    nc.vector.scalar_tensor_tensor(
                        scratch[:], dcol, -S, dcol, MULT, MULT, accum_out=bias[:]
                    )

                    dT = dtp.tile([128, D], BF16, tag="dT")
                    nc.sync.dma_start_transpose(dT[:, 0:128], dcol[:, 0:128])
                    nc.sync.dma_start_transpose(dT[:, 128:256], dcol[:, 128:256])
                    stage[rb] = (dT, bias)

                # ---- back of the pipe: matmuls, exp, c2 multiply, out DMA
                rbm = step - PRE
                if rbm >= 0:
                    dT, bias = stage.pop(rbm)
                    rs = slice(rbm * 128, (rbm + 1) * 128)
                    ps = psmm.tile([128, 2048], FP32, tag="mm")
                    for j in range(4):
                        ks = slice(j * 512, (j + 1) * 512)
                        nc.tensor.matmul(ps[:, ks], dT[:, 0:128], cT0[:, ks], start=True, stop=False)
                    for j in range(4):
                        ks = slice(j * 512, (j + 1) * 512)
                        nc.tensor.matmul(ps[:, ks], dT[:, 128:256], cT1[:, ks], start=False, stop=True)
                    praw = prawp.tile([128, K], BF16, tag="praw")
                    nc.scalar.activation(
                        praw[:], ps[:], Act.Exp, bias=bias[:], scale=2.0 * S
                    )
                    ot = rbfp.tile([128, K], BF16, tag="ot")
                    nc.vector.tensor_mul(ot[:], praw[:], e5rep[:])
                    nc.sync.dma_start(out_ap[rs, :], ot[:])

    nc.compile()
    return nc


def _get_nc():
    global _cached_nc
    if _cached_nc is None:
        _cached_nc = _build()
    return _cached_nc


def kernel(data, centers):
    data = np.ascontiguousarray(np.asarray(data, dtype=np.float32))
    centers = np.ascontiguousarray(np.asarray(centers, dtype=np.float32))
    assert data.shape == (N, D) and centers.shape == (K, D)

    data16 = data.astype(ml_dtypes.bfloat16)
    cent16 = centers.astype(ml_dtypes.bfloat16)

    nc = _get_nc()
    in_maps = [
        {"data": data16[i * N_LOC:(i + 1) * N_LOC], "centers": cent16}
        for i in range(NCORES)
    ]
    res = bass_utils.run_bass_kernel_spmd(nc, in_maps, core_ids=list(range(NCORES)))

    out = np.empty((N, OUT_W), dtype=np.float32)
    out[:, 0] = 1.0
    out[:, 1:1 + D] = data
    for i in range(NCORES):
        out[i * N_LOC:(i + 1) * N_LOC, 1 + D:] = res.results[i]["rbf"].astype(
            np.float32
        )
    return out


# revision 3
# speedup vs baseline: 2.5288x; 2.5288x over previous
"""Trainium2 Bass kernel for DicGaussianRBF.

out = concat([ones(N,1), data, exp(-5 * ||data - centers||^2)], axis=-1)
with data [65536, 256] f32, centers [2048, 256] f32 -> out [65536, 2305] f32.

Data-parallel over N across 8 NeuronCores; centers replicated. The device
computes only the RBF block [N/8, K] in bf16; the host assembles the final
f32 output (ones column and data pass-through are pure input marshaling).
Inputs are pre-cast to bf16 on the host: the device matmul consumes bf16
anyway, so uploading bf16 halves the HBM read without changing numerics.

Per core (8192 rows, 64 row-blocks of 128, 8 super-blocks of 8 rbs):

  setup: centers staged once, then ONE batched DMA-xbar transpose
  [128, 4096] -> 32 blocks of [d-chunk, k-tile]; DVE deinterleaves into
  cT0/cT1 [128, 2048] (k-contiguous). c2 = ||c||^2 via ones-matmul of
  cT^2; e5rep = exp(-5*c2) replicated to [128, K] via a PE broadcast.

  steady state: input staged one super-block (512 KB bf16) per SWDGE DMA;
  one batched xbar transpose per super-block ([128, 2048] -> 16 blocks of
  [d-chunk, n]), issued half a super-block late so its semaphore wait
  never blocks the sync queue head. Per row-block: DVE computes
  bias = -5*||x||^2 in one scalar_tensor_tensor; 8 matmuls (2 contraction
  chunks x 4 psum banks) accumulate psum = x.c over a [128, 2048] 4-bank
  psum tile; ScalarE evaluates exp(10*psum + bias) at FD=2048 into bf16;
  DVE multiplies by e5rep; HWDGE DMAs the row-block out.

Factorization note: exp(-5r^2) = exp(10xc - 5x^2) * exp(-5c2). The first
factor can overflow f32 only if 10xc - 5x^2 > 88, which requires some
||c||^2 > 17.6 with x aligned to c; for such adversarial inputs the fused
single-exp form is more robust. For data in the reference's regime the
margin is > e^40.
"""

import sys

for _p in ("/opt/trn_rl_repo",):
    if _p not in sys.path:
        sys.path.insert(0, _p)

import numpy as np
import ml_dtypes

import concourse.bass as bass
import concourse.tile as tile
from concourse import bacc, mybir
from concourse import bass_utils

N, D, K = 65536, 256, 2048
NCORES = 8
N_LOC = N // NCORES          # 8192 rows per core
OUT_W = 1 + D + K            # 2305
RB = N_LOC // 128            # 64 row blocks per core
SB = 8                       # row blocks per input staging DMA
NSB = RB // SB               # 8 super blocks
PRE = 2                      # bias pipeline lookahead (row blocks)
S = 5.0

FP32 = mybir.dt.float32
BF16 = mybir.dt.bfloat16
Act = mybir.ActivationFunctionType
MULT = mybir.AluOpType.mult

_cached_nc = None


def _build():
    nc = bacc.Bacc(
        "TRN2",
        target_bir_lowering=False,
        debug=False,
        enable_asserts=False,
        num_devices=NCORES,
    )
    data_ap = nc.dram_tensor("data", [N_LOC, D], BF16, kind="ExternalInput").ap()
    cent_ap = nc.dram_tensor("centers", [K, D], BF16, kind="ExternalInput").ap()
    out_ap = nc.dram_tensor("rbf", [N_LOC, K], BF16, kind="ExternalOutput").ap()

    with tile.TileContext(nc) as tc:
        with (
            tc.tile_pool(name="const", bufs=1) as const,
            tc.tile_pool(name="cload", bufs=1) as cload,
            tc.tile_pool(name="dinp", bufs=4) as dinp,
            tc.tile_pool(name="dtsb", bufs=3) as dtsb,
            tc.tile_pool(name="rbfp", bufs=3) as rbfp,
            tc.tile_pool(name="prawp", bufs=3) as prawp,
            tc.tile_pool(name="scrp", bufs=3) as scrp,
            tc.tile_pool(name="biasp", bufs=8) as biasp,
            tc.tile_pool(name="psmm", bufs=2, space="PSUM") as psmm,
        ):
            ones_col = const.tile([128, 1], BF16)
            nc.vector.memset(ones_col[:], 1.0)
            ones_row = const.tile([1, 128], BF16)
            nc.vector.memset(ones_row[:], 1.0)
            warm = const.tile([128, 512], BF16)
            nc.vector.memset(warm[:], 0.0)

            # dummy matmuls engage the PE HAM clock-gate so the real matmul
            # stream starts at 2.4 GHz
            pw = psmm.tile([128, 2048], FP32, tag="mm", name="pw")

            def warm_mms(n):
                for _ in range(n):
                    nc.tensor.matmul(pw[0:1, 0:512], ones_col[:], warm[:], start=True, stop=True)

            # centers staged as [128, 16, 256] (16 k-tiles of 128)
            cstage = cload.tile([128, (K // 128) * D], BF16)
            cst3 = cstage[:].rearrange("p (t d) -> p t d", d=D)

            din_tiles = {}
            dt_tiles = {}

            def load_super_block(sb):
                din = dinp.tile([128, SB * D], BF16, tag="din", name="din")
                din_tiles[sb] = din
                din3 = din[:].rearrange("p (r c) -> p r c", c=D)
                src = data_ap[sb * SB * 128:(sb + 1) * SB * 128, :].rearrange(
                    "(r p) d -> p r d", p=128
                )
                nc.gpsimd.dma_start(din3[:, :, :], src)

            def transpose_super_block(sb):
                # one batched xbar transpose: [128 n, 2048 (r,d)] ->
                # dT[p, c, f] = din[f, c*128+p], i.e. block c = 2r+chunk
                # holds [d-chunk, n] for row-block r
                din = din_tiles[sb]
                dT = dtsb.tile([128, SB * D], BF16, tag="dT", name="dT")
                dt_tiles[sb] = dT
                dt3 = dT[:].rearrange("p (c f) -> p c f", f=128)
                nc.sync.dma_start_transpose(dt3, din[:])

            for c in range(4):
                csrc = cent_ap[c * 512:(c + 1) * 512, :].rearrange(
                    "(t p) d -> p t d", p=128
                )
                nc.scalar.dma_start(cst3[:, c * 4:(c + 1) * 4, :], csrc)
                if c == 0:
                    load_super_block(0)
            load_super_block(1)

            # cT via one batched xbar transpose + DVE deinterleave.
            # ctall[p, c, f] = cstage[f, c*128+p]: block c = 2t+chunk holds
            # [d-chunk, k-tile t].
            ctall = cload.tile([128, (K // 128) * D], BF16, name="ctall")
            ctall3 = ctall[:].rearrange("p (c f) -> p c f", f=128)
            nc.sync.dma_start_transpose(ctall3, cstage[:])
            ct4 = ctall[:].rearrange("p (t two f) -> p t two f", two=2, f=128)
            cT0 = const.tile([128, K], BF16)
            cT1 = const.tile([128, K], BF16)
            nc.vector.tensor_copy(cT0[:].rearrange("p (t f) -> p t f", f=128), ct4[:, :, 0, :])
            nc.vector.tensor_copy(cT1[:].rearrange("p (t f) -> p t f", f=128), ct4[:, :, 1, :])

            transpose_super_block(0)
            transpose_super_block(1)
            warm_mms(10)

            # c2 = ||c||^2 row via ones-matmul of cT^2; e5row = exp(-5*c2)
            sq0 = cload.tile([128, K], BF16, name="sq0")
            sq1 = cload.tile([128, K], BF16, name="sq1")
            nc.vector.tensor_mul(sq0[:], cT0[:], cT0[:])
            nc.vector.tensor_mul(sq1[:], cT1[:], cT1[:])
            pc2 = psmm.tile([128, 2048], FP32, tag="mm", name="pc2")
            for j in range(4):
                ks = slice(j * 512, (j + 1) * 512)
                nc.tensor.matmul(pc2[0:1, ks], ones_col[:], sq0[:, ks], start=True, stop=False)
                nc.tensor.matmul(pc2[0:1, ks], ones_col[:], sq1[:, ks], start=False, stop=True)
            e5row = const.tile([1, K], BF16)
            nc.scalar.activation(e5row[:], pc2[0:1, :], Act.Exp, scale=-S)

            # replicate exp(-5*c2) across 128 partitions via PE broadcast
            pb = psmm.tile([128, 2048], FP32, tag="mm", name="pb")
            for j in range(4):
                ks = slice(j * 512, (j + 1) * 512)
                nc.tensor.matmul(pb[:, ks], ones_row[:], e5row[0:1, ks], start=True, stop=True)
            e5rep = const.tile([128, K], BF16)
            nc.vector.tensor_copy(e5rep[:], pb[:])

            stage = {}
            for step in range(RB + PRE):
                # ---- front of the pipe: stage input, transpose, bias
                rb = step
                if rb < RB:
                    if rb % SB == 0 and rb // SB + 2 < NSB:
                        load_super_block(rb // SB + 2)
                    if rb % SB == SB // 2 and rb // SB + 2 < NSB:
                        # issue the batched transpose well after its input
                        # load so the sync-queue head never waits on it
                        transpose_super_block(rb // SB + 2)
                    din = din_tiles[rb // SB]
                    b = rb % SB
                    dcol = din[:, b * D:(b + 1) * D]

                    scratch = scrp.tile([128, D], BF16, tag="scr")
                    bias = biasp.tile([128, 1], FP32, tag="bias")
                    nc.vector.scalar_tensor_tensor(
                        scratch[:], dcol, -S, dcol, MULT, MULT, accum_out=bias[:]
                    )
                    stage[rb] = bias

                # ---- back of the pipe: matmuls, exp, c2 multiply, out DMA
                rbm = step - PRE
                if rbm >= 0:
                    bias = stage.pop(rbm)
                    dT = dt_tiles[rbm // SB]
                    b = rbm % SB
                    lhs0 = dT[:, (2 * b) * 128:(2 * b + 1) * 128]
                    lhs1 = dT[:, (2 * b + 1) * 128:(2 * b + 2) * 128]
                    rs = slice(rbm * 128, (rbm + 1) * 128)
                    ps = psmm.tile([128, 2048], FP32, tag="mm")
                    for j in range(4):
                        ks = slice(j * 512, (j + 1) * 512)
                        nc.tensor.matmul(ps[:, ks], lhs0, cT0[:, ks], start=True, stop=False)
                    for j in range(4):
                        ks = slice(j * 512, (j + 1) * 512)
                        nc.tensor.matmul(ps[:, ks], lhs1, cT1[:, ks], start=False, stop=True)
                    praw = prawp.tile([128, K], BF16, tag="praw")
                    nc.scalar.activation(
                        praw[:], ps[:], Act.Exp, bias=bias[:], scale=2.0 * S
                    )
                    ot = rbfp.tile([128, K], BF16, tag="ot")
                    nc.vector.tensor_mul(ot[:], praw[:], e5rep[:])
                    nc.sync.dma_start(out_ap[rs, :], ot[:])

    nc.compile()
    return nc


def _get_nc():
    global _cached_nc
    if _cached_nc is None:
        _cached_nc = _build()
    return _cached_nc


def kernel(data, centers):
    data = np.ascontiguousarray(np.asarray(data, dtype=np.float32))
    centers = np.ascontiguousarray(np.asarray(centers, dtype=np.float32))
    assert data.shape == (N, D) and centers.shape == (K, D)

    data16 = data.astype(ml_dtypes.bfloat16)
    cent16 = centers.astype(ml_dtypes.bfloat16)

    nc = _get_nc()
    in_maps = [
        {"data": data16[i * N_LOC:(i + 1) * N_LOC], "centers": cent16}
        for i in range(NCORES)
    ]
    res = bass_utils.run_bass_kernel_spmd(nc, in_maps, core_ids=list(range(NCORES)))

    out = np.empty((N, OUT_W), dtype=np.float32)
    out[:, 0] = 1.0
    out[:, 1:1 + D] = data
    for i in range(NCORES):
        out[i * N_LOC:(i + 1) * N_LOC, 1 + D:] = res.results[i]["rbf"].astype(
            np.float32
        )
    return out
